# revision 6
# baseline (speedup 1.0000x reference)
"""Bass/Trainium2 kernel for a 2-layer bidirectional QRNN (fo-pooling).

Reference computation (per layer, per direction):
    ZFO = X @ W + b            # [S, B, 3H]
    Z, F, O = split(ZFO); Z = tanh(Z); F = sigmoid(F); O = sigmoid(O)
    c_t = F_t * c_{t-1} + (1 - F_t) * Z_t        (bw direction: reversed time)
    Y_dir = O * C
    Y = concat(Y_fw, Y_bw)     # [S, B, 2H]
Two stacked layers; output is [B, S, 2H].

Sharding: data-parallel over batch. B=16 rows -> 2 rows per NeuronCore x 8.
Each core runs both layers for its 2 rows; no collectives.

Device layout: everything is kept feature-major ([feat, seq] per batch row) so
the matmul (which contracts over the partition axis) needs no on-chip
transposes: layer-0 input is host-pre-transposed X^T, layer-0 output Y1 is
produced feature-major (exactly what layer 1 consumes), and the final output
is un-transposed on the host.

The time recurrence uses the DVE `tensor_tensor_scan` instruction
(state = f*state + g along the free axis); the bw direction runs the scan
through reversed access patterns with s-tiles processed in descending order.
"""

import numpy as np

import concourse.bacc as bacc
import concourse.bass as bass
import concourse.mybir as mybir
from concourse import bass_utils
from concourse.tile import TileContext

# problem dims (hardcoded per spec)
B, S, D, H = 16, 2048, 512, 512
N_CORES = 8
BC = B // N_CORES  # batch rows per core
P = 128  # SBUF partitions
S_TILE = 512

F32 = mybir.dt.float32
ACT = mybir.ActivationFunctionType
ALU = mybir.AluOpType


def build_nc(bc=BC, s=S, d=D, h=H, s_tile=S_TILE, mm_dtype="fp32"):
    """Build the SPMD Bass program (same program on every core)."""
    nc = bacc.Bacc("TRN2", target_bir_lowering=False)

    xt = nc.dram_tensor("xt", [bc, d, s], F32, kind="ExternalInput")
    w0f = nc.dram_tensor("w0f", [d, 3 * h], F32, kind="ExternalInput")
    w0b = nc.dram_tensor("w0b", [d, 3 * h], F32, kind="ExternalInput")
    b0f = nc.dram_tensor("b0f", [3 * h], F32, kind="ExternalInput")
    b0b = nc.dram_tensor("b0b", [3 * h], F32, kind="ExternalInput")
    w1f = nc.dram_tensor("w1f", [2 * h, 3 * h], F32, kind="ExternalInput")
    w1b = nc.dram_tensor("w1b", [2 * h, 3 * h], F32, kind="ExternalInput")
    b1f = nc.dram_tensor("b1f", [3 * h], F32, kind="ExternalInput")
    b1b = nc.dram_tensor("b1b", [3 * h], F32, kind="ExternalInput")
    y1 = nc.dram_tensor("y1", [bc, 2 * h, s], F32)  # layer-0 out / layer-1 in
    out_t = nc.dram_tensor("out_t", [bc, 2 * h, s], F32, kind="ExternalOutput")

    ns = s // s_tile
    hc = h // P

    def mmcast(ap):
        if mm_dtype == "fp32r":
            return ap.bitcast(mybir.dt.float32r)
        return ap

    def layer_pass(tc, layer, fw, wd, bd, in_tiles, dst):
        """One (layer, direction) pass over all batch rows.

        in_tiles: for layer 0, dict (b, k) -> resident SBUF tile [P, s];
                  None for layer 1 (streams y1 from DRAM).
        dst: DRAM destination ([bc, 2h, s]); this pass writes rows
             [dir_off : dir_off + h] where dir_off = 0 (fw) or h (bw).
        """
        k_chunks = (d if layer == 0 else 2 * h) // P
        dir_off = 0 if fw else h
        tag = f"L{layer}{'f' if fw else 'b'}"
        with (
            tc.tile_pool(name=f"w_{tag}", bufs=1) as wpool,
            tc.tile_pool(name=f"scr_{tag}", bufs=3) as spool,
            tc.tile_pool(name=f"carry_{tag}", bufs=1) as cpool,
            tc.tile_pool(name=f"in_{tag}", bufs=2) as ypool,
            tc.tile_pool(name=f"ps_{tag}", bufs=2, space="PSUM") as ppool,
        ):
            # weights: one [P, 3h] tile per contraction chunk
            wk = []
            for k in range(k_chunks):
                t = wpool.tile([P, 3 * h], F32, tag=f"wk{k}")
                nc.sync.dma_start(t[:], wd[k * P : (k + 1) * P, :])
                wk.append(t)
            # per-(gate, h-chunk) bias columns [P, 1]
            btile = {}
            for g in range(3):
                for hh in range(hc):
                    t = wpool.tile([P, 1], F32, tag=f"b{g}_{hh}")
                    nc.sync.dma_start(
                        t[:], bd[g * h + hh * P : g * h + (hh + 1) * P].unsqueeze(1)
                    )
                    btile[g, hh] = t

            s_order = list(range(ns)) if fw else list(range(ns - 1, -1, -1))
            for b in range(bc):
                carry = [cpool.tile([P, 1], F32, tag=f"c{hh}", name=f"carry{hh}") for hh in range(hc)]
                for si, s_idx in enumerate(s_order):
                    s0 = s_idx * s_tile
                    ins = []
                    for k in range(k_chunks):
                        if layer == 0:
                            ins.append(in_tiles[b, k][:, s0 : s0 + s_tile])
                        else:
                            t = ypool.tile([P, s_tile], F32, tag=f"in{k}")
                            nc.sync.dma_start(
                                t[:], y1[b, k * P : (k + 1) * P, s0 : s0 + s_tile]
                            )
                            ins.append(t[:])
                    for hh in range(hc):
                        ps = [
                            ppool.tile([P, s_tile], F32, tag=f"ps{g}", name=f"ps{g}")
                            for g in range(3)
                        ]
                        for g in range(3):
                            cols = slice(g * h + hh * P, g * h + (hh + 1) * P)
                            for k in range(k_chunks):
                                nc.tensor.matmul(
                                    ps[g][:],
                                    mmcast(wk[k][:, cols]),
                                    mmcast(ins[k]),
                                    start=(k == 0),
                                    stop=(k == k_chunks - 1),
                                )
                        z = spool.tile([P, s_tile], F32, tag="z")
                        f_ = spool.tile([P, s_tile], F32, tag="f")
                        o = spool.tile([P, s_tile], F32, tag="o")
                        fn = spool.tile([P, s_tile], F32, tag="fn")
                        g_ = spool.tile([P, s_tile], F32, tag="g")
                        c = spool.tile([P, s_tile], F32, tag="c")
                        y = spool.tile([P, s_tile], F32, tag="y")
                        nc.scalar.activation(z[:], ps[0][:], ACT.Tanh, bias=btile[0, hh][:])
                        nc.scalar.activation(f_[:], ps[1][:], ACT.Sigmoid, bias=btile[1, hh][:])
                        nc.scalar.activation(o[:], ps[2][:], ACT.Sigmoid, bias=btile[2, hh][:])
                        # g = (1 - f) * z
                        nc.vector.tensor_scalar(fn[:], f_[:], -1.0, 1.0, ALU.mult, ALU.add)
                        nc.vector.tensor_mul(g_[:], fn[:], z[:])
                        # c_t = f_t * c_prev + g_t (bw: time runs backwards)
                        if fw:
                            sc = (c[:], f_[:], g_[:])
                            carry_col = slice(s_tile - 1, s_tile)
                        else:
                            sc = (c[:, ::-1], f_[:, ::-1], g_[:, ::-1])
                            carry_col = slice(0, 1)
                        init = 0.0 if si == 0 else carry[hh][:]
                        nc.vector.tensor_tensor_scan(
                            sc[0], sc[1], sc[2], init, ALU.mult, ALU.add
                        )
                        if si < ns - 1:
                            nc.vector.tensor_copy(carry[hh][:], c[:, carry_col])
                        nc.vector.tensor_mul(y[:], o[:], c[:])
                        row0 = dir_off + hh * P
                        nc.sync.dma_start(
                            dst[b, row0 : row0 + P, s0 : s0 + s_tile], y[:]
                        )

    with TileContext(nc) as tc:
        # ---- layer 0: X^T resident in SBUF
        with tc.tile_pool(name="xres", bufs=1) as xpool:
            xtiles = {}
            for b in range(bc):
                for k in range(d // P):
                    t = xpool.tile([P, s], F32, tag=f"x_{b}_{k}")
                    nc.sync.dma_start(t[:], xt[b, k * P : (k + 1) * P, :])
                    xtiles[b, k] = t
            layer_pass(tc, 0, True, w0f, b0f, xtiles, y1)
            layer_pass(tc, 0, False, w0b, b0b, xtiles, y1)
        # ---- layer 1: streams y1, writes output
        layer_pass(tc, 1, True, w1f, b1f, None, out_t)
        layer_pass(tc, 1, False, w1b, b1b, None, out_t)

    nc.finalize()
    return nc


_NC_CACHE = {}


def _get_nc(mm_dtype):
    if mm_dtype not in _NC_CACHE:
        _NC_CACHE[mm_dtype] = build_nc(mm_dtype=mm_dtype)
    return _NC_CACHE[mm_dtype]


def kernel(X, seqlens, W_fw0, b_fw0, W_bw0, b_bw0, W_fw1, b_fw1, W_bw1, b_bw1,
           mm_dtype="fp32", trace=False):
    """Full-input entry point: shards over 8 cores, returns [B, S, 2H] f32."""
    del seqlens  # unused by the reference computation
    X = np.ascontiguousarray(np.asarray(X, dtype=np.float32))
    weights = {
        "w0f": W_fw0, "b0f": b_fw0, "w0b": W_bw0, "b0b": b_bw0,
        "w1f": W_fw1, "b1f": b_fw1, "w1b": W_bw1, "b1b": b_bw1,
    }
    weights = {k: np.ascontiguousarray(np.asarray(v, dtype=np.float32))
               for k, v in weights.items()}

    nc = _get_nc(mm_dtype)
    in_maps = []
    for i in range(N_CORES):
        rows = X[i * BC : (i + 1) * BC]  # [BC, S, D]
        xt_i = np.ascontiguousarray(rows.transpose(0, 2, 1))  # [BC, D, S]
        in_maps.append({"xt": xt_i, **weights})

    res = bass_utils.run_bass_kernel_spmd(
        nc, in_maps, core_ids=list(range(N_CORES)), trace=trace
    )
    out = np.empty((B, S, 2 * H), dtype=np.float32)
    for i in range(N_CORES):
        out_t = res.results[i]["out_t"]  # [BC, 2H, S]
        out[i * BC : (i + 1) * BC] = out_t.transpose(0, 2, 1)
    kernel.last_results = res
    return out


# revision 11
# speedup vs baseline: 2.9372x; 2.9372x over previous
"""Bass/Trainium2 kernel for a 2-layer bidirectional QRNN (fo-pooling).

Reference computation (per layer, per direction):
    ZFO = X @ W + b            # [S, B, 3H]
    Z, F, O = split(ZFO); Z = tanh(Z); F = sigmoid(F); O = sigmoid(O)
    c_t = F_t * c_{t-1} + (1 - F_t) * Z_t        (bw direction: reversed time)
    Y_dir = O * C
    Y = concat(Y_fw, Y_bw)     # [S, B, 2H]
Two stacked layers; output is [B, S, 2H].

Sharding: data-parallel over batch. B=16 rows -> 2 rows per NeuronCore x 8.
Each core runs both layers for its 2 rows; no collectives.

Device layout: everything is kept feature-major ([feat, seq] per batch row) so
the matmul (which contracts over the partition axis) needs no on-chip
transposes: layer-0 input is host-pre-transposed X^T, layer-0 output Y1 is
produced feature-major (exactly what layer 1 consumes), and the final output
is un-transposed on the host.

The time recurrence uses the DVE `tensor_tensor_scan` instruction
(state = f*state + g along the free axis); the bw direction runs the scan
through reversed access patterns with s-tiles processed in descending order.
"""

import numpy as np

import concourse.bacc as bacc
import concourse.bass as bass
import concourse.mybir as mybir
from concourse import bass_utils
from concourse.tile import TileContext

# problem dims (hardcoded per spec)
B, S, D, H = 16, 2048, 512, 512
N_CORES = 8
BC = B // N_CORES  # batch rows per core
P = 128  # SBUF partitions
S_TILE = 512

F32 = mybir.dt.float32
ACT = mybir.ActivationFunctionType
ALU = mybir.AluOpType


def build_nc(bc=BC, s=S, d=D, h=H, s_tile=S_TILE, mm_dtype="fp32"):
    """Build the SPMD Bass program (same program on every core)."""
    nc = bacc.Bacc("TRN2", target_bir_lowering=False)

    xt = nc.dram_tensor("xt", [bc, d, s], F32, kind="ExternalInput")
    w0f = nc.dram_tensor("w0f", [d, 3 * h], F32, kind="ExternalInput")
    w0b = nc.dram_tensor("w0b", [d, 3 * h], F32, kind="ExternalInput")
    b0f = nc.dram_tensor("b0f", [3 * h], F32, kind="ExternalInput")
    b0b = nc.dram_tensor("b0b", [3 * h], F32, kind="ExternalInput")
    w1f = nc.dram_tensor("w1f", [2 * h, 3 * h], F32, kind="ExternalInput")
    w1b = nc.dram_tensor("w1b", [2 * h, 3 * h], F32, kind="ExternalInput")
    b1f = nc.dram_tensor("b1f", [3 * h], F32, kind="ExternalInput")
    b1b = nc.dram_tensor("b1b", [3 * h], F32, kind="ExternalInput")
    y1 = nc.dram_tensor("y1", [bc, 2 * h, s], F32)  # layer-0 out / layer-1 in
    out_t = nc.dram_tensor("out_t", [bc, 2 * h, s], F32, kind="ExternalOutput")

    ns = s // s_tile
    hc = h // P
    # fp32r (TF32-like 10-bit-mantissa) matmul inputs run the PE at 4x the
    # fp32 rate. fp32r tiles must be produced by a compute-engine cast (a
    # fp32r DMA faults the exec unit; a bitcast fails BIR verification).
    mmdt = mybir.dt.float32r if mm_dtype == "fp32r" else F32

    def layer_pass(tc, layer, fw, wd, bd, in_tiles, dst):
        """One (layer, direction) pass over all batch rows.

        in_tiles: for layer 0, dict (b, k) -> resident SBUF tile [P, s];
                  None for layer 1 (streams y1 from DRAM).
        dst: DRAM destination ([bc, 2h, s]); this pass writes rows
             [dir_off : dir_off + h] where dir_off = 0 (fw) or h (bw).
        """
        k_chunks = (d if layer == 0 else 2 * h) // P
        dir_off = 0 if fw else h
        tag = f"L{layer}{'f' if fw else 'b'}"
        with (
            tc.tile_pool(name=f"w_{tag}", bufs=1) as wpool,
            tc.tile_pool(name=f"scr_{tag}", bufs=3) as spool,
            tc.tile_pool(name=f"carry_{tag}", bufs=1) as cpool,
            tc.tile_pool(name=f"in_{tag}", bufs=2) as ypool,
            tc.tile_pool(name=f"ps_{tag}", bufs=2, space="PSUM") as ppool,
        ):
            # weights: one [P, 3h] tile per contraction chunk
            wk = []
            for k in range(k_chunks):
                if mmdt is F32:
                    t = wpool.tile([P, 3 * h], F32, tag=f"wk{k}")
                    nc.sync.dma_start(t[:], wd[k * P : (k + 1) * P, :])
                else:
                    stg = ypool.tile([P, 3 * h], F32, tag="wstg", name="wstg")
                    nc.sync.dma_start(stg[:], wd[k * P : (k + 1) * P, :])
                    t = wpool.tile([P, 3 * h], mmdt, tag=f"wk{k}")
                    nc.scalar.copy(t[:], stg[:])
                wk.append(t)
            # per-(gate, h-chunk) bias columns [P, 1]
            btile = {}
            for g in range(3):
                for hh in range(hc):
                    t = wpool.tile([P, 1], F32, tag=f"b{g}_{hh}")
                    nc.sync.dma_start(
                        t[:], bd[g * h + hh * P : g * h + (hh + 1) * P].unsqueeze(1)
                    )
                    btile[g, hh] = t

            s_order = list(range(ns)) if fw else list(range(ns - 1, -1, -1))
            for b in range(bc):
                carry = [cpool.tile([P, 1], F32, tag=f"c{hh}", name=f"carry{hh}") for hh in range(hc)]
                for si, s_idx in enumerate(s_order):
                    s0 = s_idx * s_tile
                    ins = []
                    for k in range(k_chunks):
                        if layer == 0:
                            ins.append(in_tiles[b, k][:, s0 : s0 + s_tile])
                        else:
                            t = ypool.tile([P, s_tile], F32, tag=f"in{k}")
                            nc.sync.dma_start(
                                t[:], y1[b, k * P : (k + 1) * P, s0 : s0 + s_tile]
                            )
                            if mmdt is not F32:
                                tr = ypool.tile([P, s_tile], mmdt, tag=f"inr{k}", name=f"inr{k}")
                                nc.vector.tensor_copy(tr[:], t[:])
                                t = tr
                            ins.append(t[:])
                    for hh in range(hc):
                        ps = [
                            ppool.tile([P, s_tile], F32, tag=f"ps{g}", name=f"ps{g}")
                            for g in range(3)
                        ]
                        for g in range(3):
                            cols = slice(g * h + hh * P, g * h + (hh + 1) * P)
                            for k in range(k_chunks):
                                nc.tensor.matmul(
                                    ps[g][:],
                                    wk[k][:, cols],
                                    ins[k],
                                    start=(k == 0),
                                    stop=(k == k_chunks - 1),
                                )
                        z = spool.tile([P, s_tile], F32, tag="z")
                        f_ = spool.tile([P, s_tile], F32, tag="f")
                        o = spool.tile([P, s_tile], F32, tag="o")
                        fn = spool.tile([P, s_tile], F32, tag="fn")
                        g_ = spool.tile([P, s_tile], F32, tag="g")
                        c = spool.tile([P, s_tile], F32, tag="c")
                        y = spool.tile([P, s_tile], F32, tag="y")
                        nc.scalar.activation(z[:], ps[0][:], ACT.Tanh, bias=btile[0, hh][:])
                        nc.scalar.activation(f_[:], ps[1][:], ACT.Sigmoid, bias=btile[1, hh][:])
                        nc.scalar.activation(o[:], ps[2][:], ACT.Sigmoid, bias=btile[2, hh][:])
                        # g = (1 - f) * z
                        nc.vector.tensor_scalar(fn[:], f_[:], -1.0, 1.0, ALU.mult, ALU.add)
                        nc.vector.tensor_mul(g_[:], fn[:], z[:])
                        # c_t = f_t * c_prev + g_t (bw: time runs backwards)
                        if fw:
                            sc = (c[:], f_[:], g_[:])
                            carry_col = slice(s_tile - 1, s_tile)
                        else:
                            sc = (c[:, ::-1], f_[:, ::-1], g_[:, ::-1])
                            carry_col = slice(0, 1)
                        init = 0.0 if si == 0 else carry[hh][:]
                        nc.vector.tensor_tensor_scan(
                            sc[0], sc[1], sc[2], init, ALU.mult, ALU.add
                        )
                        if si < ns - 1:
                            nc.vector.tensor_copy(carry[hh][:], c[:, carry_col])
                        nc.vector.tensor_mul(y[:], o[:], c[:])
                        row0 = dir_off + hh * P
                        nc.sync.dma_start(
                            dst[b, row0 : row0 + P, s0 : s0 + s_tile], y[:]
                        )

    with TileContext(nc) as tc:
        # ---- layer 0: X^T resident in SBUF
        with tc.tile_pool(name="xres", bufs=1) as xpool:
            xtiles = {}
            with tc.tile_pool(name="xstage", bufs=2) as xstage:
                for b in range(bc):
                    for k in range(d // P):
                        if mmdt is F32:
                            t = xpool.tile([P, s], F32, tag=f"x_{b}_{k}")
                            nc.sync.dma_start(t[:], xt[b, k * P : (k + 1) * P, :])
                        else:
                            stg = xstage.tile([P, s], F32, tag="xstg", name="xstg")
                            nc.sync.dma_start(stg[:], xt[b, k * P : (k + 1) * P, :])
                            t = xpool.tile([P, s], mmdt, tag=f"x_{b}_{k}")
                            nc.scalar.copy(t[:], stg[:])
                        xtiles[b, k] = t
            layer_pass(tc, 0, True, w0f, b0f, xtiles, y1)
            layer_pass(tc, 0, False, w0b, b0b, xtiles, y1)
        # ---- layer 1: streams y1, writes output
        layer_pass(tc, 1, True, w1f, b1f, None, out_t)
        layer_pass(tc, 1, False, w1b, b1b, None, out_t)

    nc.finalize()
    return nc


_NC_CACHE = {}


def _get_nc(mm_dtype):
    if mm_dtype not in _NC_CACHE:
        _NC_CACHE[mm_dtype] = build_nc(mm_dtype=mm_dtype)
    return _NC_CACHE[mm_dtype]


def kernel(X, seqlens, W_fw0, b_fw0, W_bw0, b_bw0, W_fw1, b_fw1, W_bw1, b_bw1,
           mm_dtype="fp32", trace=False):
    """Full-input entry point: shards over 8 cores, returns [B, S, 2H] f32."""
    del seqlens  # unused by the reference computation
    X = np.ascontiguousarray(np.asarray(X, dtype=np.float32))
    weights = {
        "w0f": W_fw0, "b0f": b_fw0, "w0b": W_bw0, "b0b": b_bw0,
        "w1f": W_fw1, "b1f": b_fw1, "w1b": W_bw1, "b1b": b_bw1,
    }
    weights = {k: np.ascontiguousarray(np.asarray(v, dtype=np.float32))
               for k, v in weights.items()}

    nc = _get_nc(mm_dtype)
    in_maps = []
    for i in range(N_CORES):
        rows = X[i * BC : (i + 1) * BC]  # [BC, S, D]
        xt_i = np.ascontiguousarray(rows.transpose(0, 2, 1))  # [BC, D, S]
        in_maps.append({"xt": xt_i, **weights})

    res = bass_utils.run_bass_kernel_spmd(
        nc, in_maps, core_ids=list(range(N_CORES)), trace=trace
    )
    out = np.empty((B, S, 2 * H), dtype=np.float32)
    for i in range(N_CORES):
        out_t = res.results[i]["out_t"]  # [BC, 2H, S]
        out[i * BC : (i + 1) * BC] = out_t.transpose(0, 2, 1)
    kernel.last_results = res
    return out
